# revision 15
# baseline (speedup 1.0000x reference)
"""HONU order-3 kernel for 8 TRN2 NeuronCores.

Math: out[b] = sum_{i<=j<=k} w_ijk * xf_i * xf_j * xf_k,  xf = [1, x] (127 feats).

Restructuring: group combos by pair (i,j) (lex order => per-pair weights are a
contiguous slice of `weights`).  Let W[(i,j), k] = w_ijk for k>=j (0 otherwise).
Then  Z[b,(i,j)] = sum_k W[(i,j),k] * xf[b,k]   (a dense matmul), and with the
host-precomputed pair products P[b,(i,j)] = xf_i[b] * xf_j[b]:
      out[b]     = sum_{(i,j)} P[b,(i,j)] * Z[b,(i,j)]
i.e. three wide fused multiply-accumulate DVE ops per 128-batch tile instead of
one narrow op per i-row.

Sharding: pair-rows i are dealt round-robin to the 8 cores (core c gets rows
i = 8t + c, t = 0..15); every core runs the same (SPMD) program over its 1088
padded pair-columns.

Layout/latency choices (from NTFF traces):
- All input DMAs ride ONE hardware queue (Sync) in priority order; a single
  queue fans out over all 16 DMA engines, so ordering beats 2-queue
  bandwidth sharing.  Order: xt|wd (gates the matmuls), then P chunk-major
  (both batch tiles of chunk c fused per DMA -> fat 1856/1600/896B
  descriptors AND per-chunk semaphores so the DVE starts on chunk 0
  without waiting for the rest).
- bf16 inputs halve DMA bytes (rel err ~1.7e-3, gate is 2e-2).
- DVE reads Z straight from PSUM (no scalar-engine staging copy).
- Compute runs chunk-major (c0 tile0, c0 tile1, c1 tile0, ...) to follow
  the DMA arrival order.
- The [128,2] result is transposed on the PE (identity built on-chip by
  GpSimd) so the output store is a single 2-descriptor DMA instead of 128
  4-byte descriptors whose serialized semaphore updates cost ~8us of tail.
Host sums the 8 per-core [2,128] partials.
"""

import numpy as np

import concourse.bass as bass
import concourse.bacc as bacc
import concourse.tile as tile
import concourse.mybir as mybir
from concourse.bass_utils import run_bass_kernel_spmd
from concourse.masks import make_identity

F32 = mybir.dt.float32
BF16 = mybir.dt.bfloat16
NP_BF16 = mybir.dt.np(BF16)

P = 128
NF = 127            # features incl. bias
B = 256             # batch
NCLASS = 16         # width classes (i-rows per core)
WIDTHS = [128 - 8 * t for t in range(NCLASS)]            # 128,120,...,8
OFFS = np.concatenate([[0], np.cumsum(WIDTHS)])          # class col offsets
NCOLS = int(OFFS[-1])                                    # 1088
# chunk = (class range); each chunk is one matmul (N<=512)
CHUNKS = [(0, 4), (4, 9), (9, 16)]
CHUNK_COLS = [int(OFFS[hi] - OFFS[lo]) for lo, hi in CHUNKS]  # 464, 400, 224
# pp DRAM layout: chunk-major, tiles paired: [c0t0|c0t1|c1t0|c1t1|c2t0|c2t1]
PP_OFFS = np.concatenate([[0], np.cumsum([2 * n for n in CHUNK_COLS])])

_CACHE = {}


def _build_nc():
    nc = bacc.Bacc("TRN2", target_bir_lowering=False, debug=False)
    # xw = xt (cols 0..255) | wd (cols 256..1343), bf16
    xw = nc.dram_tensor("xw", [P, 256 + NCOLS], BF16, kind="ExternalInput")
    pp = nc.dram_tensor("pp", [P, 2 * NCOLS], BF16, kind="ExternalInput")
    out = nc.dram_tensor("out", [2, P], F32, kind="ExternalOutput")

    with tile.TileContext(nc) as tc:
        with (
            tc.tile_pool(name="sb", bufs=1) as sb,
            tc.tile_pool(name="ps", bufs=1, space="PSUM") as ps,
        ):
            # single pool per memory space: fewer teardown semaphore hops
            # between the output DMA and the final barrier (all in the
            # measured window); `s` needs no double buffer — consecutive
            # STTs on the in-order DVE can never overlap its reuse
            cpool = scr = sb
            pso = ps
            xw_t = cpool.tile([P, 256 + NCOLS], BF16, tag="xw")
            nc.sync.dma_start(xw_t[:], xw[:])
            pp_t = cpool.tile([P, 2 * NCOLS], BF16, tag="pp")
            for ci in range(3):
                o, e = int(PP_OFFS[ci]), int(PP_OFFS[ci + 1])
                nc.sync.dma_start(pp_t[:, o:e], pp[:, o:e])

            id_t = cpool.tile([P, P], BF16, tag="idm")
            make_identity(nc, id_t[:])

            g = sb.tile([P, 6], F32, tag="g")
            # bf16 res2 -> PE transpose runs at 1 cycle/row instead of 2;
            # only rounds the final per-batch sums (~0.4% rel, gate is 2e-2)
            res2 = sb.tile([P, 2], BF16, tag="res2")
            # chunk-major: follow DMA arrival order
            for ci, (lo, hi) in enumerate(CHUNKS):
                n = CHUNK_COLS[ci]
                o = int(OFFS[lo])
                for bt in range(2):
                    z_ps = ps.tile([P, 464], F32, tag=f"z{ci}{bt}",
                                   name=f"z{ci}{bt}_ps")
                    nc.tensor.matmul(
                        z_ps[:, :n],
                        xw_t[:, bt * P:(bt + 1) * P],
                        xw_t[:, 256 + o:256 + o + n],
                        start=True, stop=True,
                    )
                    s = scr.tile([P, 464], F32, tag="s", name="s_t")
                    nc.vector.scalar_tensor_tensor(
                        out=s[:, :n],
                        in0=z_ps[:, :n],
                        scalar=1.0,
                        in1=pp_t[:, int(PP_OFFS[ci]) + bt * n:
                                 int(PP_OFFS[ci]) + (bt + 1) * n],
                        op0=mybir.AluOpType.mult,
                        op1=mybir.AluOpType.mult,
                        accum_out=g[:, 3 * bt + ci:3 * bt + ci + 1],
                    )
            with nc.allow_low_precision("final per-batch sums; 0.4% << 2e-2 gate"):
                for bt in range(2):
                    nc.vector.reduce_sum(
                        res2[:, bt:bt + 1], g[:, 3 * bt:3 * bt + 3],
                        axis=mybir.AxisListType.X,
                    )
            t_ps = pso.tile([2, P], BF16, tag="t")
            nc.tensor.transpose(t_ps[:], res2[:], id_t[:])
            t_sb = sb.tile([2, P], F32, tag="tsb")
            nc.vector.tensor_copy(t_sb[:], t_ps[:])
            nc.sync.dma_start(out[:], t_sb[:])
    nc.compile()
    return nc


def _prep_inputs(x, weights, comb_idx):
    """Host-side layout prep: xf paddings, per-core dense weight chunks, and
    the pair-product matrices P[b,(i,j)] = xf_i * xf_j (bf16)."""
    x = np.ascontiguousarray(np.asarray(x, dtype=np.float32))
    w = np.asarray(weights, dtype=np.float32).ravel()
    ci = np.asarray(comb_idx)
    i_, j_ = ci[:, 0].astype(np.int64), ci[:, 1].astype(np.int64)
    k_ = ci[:, 2].astype(np.int64)

    xf = np.concatenate([np.ones((B, 1), np.float32), x], axis=1)   # [256,127]
    xt = np.zeros((P, B), np.float32)
    xt[:NF, :] = xf.T
    xt_b = xt.astype(NP_BF16)

    # lex pair-row index of each combo
    ar = np.arange(NF, dtype=np.int64)
    rsp = ar * NF - (ar * (ar - 1)) // 2
    q = rsp[i_] + (j_ - i_)
    Wd = np.zeros((8128, NF), np.float32)
    Wd[q, k_] = w

    in_maps = []
    for c in range(8):
        big = np.zeros((P, NCOLS), np.float32)
        Pp = np.zeros((B, NCOLS), np.float32)
        for t in range(NCLASS):
            i = 8 * t + c
            if i > 126:
                continue
            o = int(OFFS[t])
            wdt = WIDTHS[t]
            p0 = int(rsp[i])
            # cols j in [i,127) hold Wd rows p0..p0+(127-i); leading j in
            # [8t, i) and trailing j=127 stay zero
            big[:NF, o + (i - 8 * t): o + (127 - 8 * t)] = Wd[p0:p0 + (NF - i)].T
            # pair products for j grid 8t..min(8t+w,127)-1 (zero-weight cols:
            # value irrelevant, z=0 there)
            jhi = min(8 * t + wdt, NF)
            Pp[:, o:o + (jhi - 8 * t)] = xf[:, i:i + 1] * xf[:, 8 * t:jhi]
        Pp_b = Pp.astype(NP_BF16)
        # chunk-major, tile-paired pp layout
        pieces = []
        for cidx, (lo, hi) in enumerate(CHUNKS):
            o, e = int(OFFS[lo]), int(OFFS[hi])
            pieces.append(Pp_b[:P, o:e])
            pieces.append(Pp_b[P:, o:e])
        m = {
            "xw": np.ascontiguousarray(
                np.concatenate([xt_b, big.astype(NP_BF16)], axis=1)),
            "pp": np.ascontiguousarray(np.concatenate(pieces, axis=1)),
        }
        in_maps.append(m)
    return in_maps


def _get_nc():
    if "nc" not in _CACHE:
        _CACHE["nc"] = _build_nc()
    return _CACHE["nc"]


def run_spmd(x, weights, comb_idx, trace=False):
    nc = _get_nc()
    in_maps = _prep_inputs(x, weights, comb_idx)
    res = run_bass_kernel_spmd(nc, in_maps, list(range(8)), trace=trace)
    acc = np.zeros(B, np.float64)
    for c in range(8):
        acc += res.results[c]["out"].astype(np.float64).ravel()
    return acc.astype(np.float32)[:, None], res


def kernel(x, weights, comb_idx):
    out, _ = run_spmd(x, weights, comb_idx, trace=False)
    return out


# revision 16
# speedup vs baseline: 1.0541x; 1.0541x over previous
"""HONU order-3 kernel for 8 TRN2 NeuronCores.

Math: out[b] = sum_{i<=j<=k} w_ijk * xf_i * xf_j * xf_k,  xf = [1, x] (127 feats).

Restructuring: group combos by pair (i,j) (lex order => per-pair weights are a
contiguous slice of `weights`).  Let W[(i,j), k] = w_ijk for k>=j (0 otherwise).
Then  Z[b,(i,j)] = sum_k W[(i,j),k] * xf[b,k]   (a dense matmul), and with the
host-precomputed pair products P[b,(i,j)] = xf_i[b] * xf_j[b]:
      out[b]     = sum_{(i,j)} P[b,(i,j)] * Z[b,(i,j)]
i.e. three wide fused multiply-accumulate DVE ops per 128-batch tile instead of
one narrow op per i-row.

Sharding: pair-rows i are dealt round-robin to the 8 cores (core c gets rows
i = 8t + c, t = 0..15); every core runs the same (SPMD) program over its 1088
padded pair-columns.

Layout/latency choices (from NTFF traces):
- All input DMAs ride ONE hardware queue (Sync) in priority order; a single
  queue fans out over all 16 DMA engines, so ordering beats 2-queue
  bandwidth sharing.  Order: xt|wd (gates the matmuls), then P chunk-major
  (both batch tiles of chunk c fused per DMA -> fat 1856/1600/896B
  descriptors AND per-chunk semaphores so the DVE starts on chunk 0
  without waiting for the rest).
- bf16 inputs halve DMA bytes (rel err ~1.7e-3, gate is 2e-2).
- DVE reads Z straight from PSUM (no scalar-engine staging copy).
- Compute runs chunk-major (c0 tile0, c0 tile1, c1 tile0, ...) to follow
  the DMA arrival order.
- The [128,2] result is transposed on the PE (identity built on-chip by
  GpSimd) so the output store is a single 2-descriptor DMA instead of 128
  4-byte descriptors whose serialized semaphore updates cost ~8us of tail.
Host sums the 8 per-core [2,128] partials.
"""

import numpy as np

import concourse.bass as bass
import concourse.bacc as bacc
import concourse.tile as tile
import concourse.mybir as mybir
from concourse.bass_utils import run_bass_kernel_spmd
from concourse.masks import make_identity

F32 = mybir.dt.float32
BF16 = mybir.dt.bfloat16
NP_BF16 = mybir.dt.np(BF16)

P = 128
NF = 127            # features incl. bias
B = 256             # batch
NCLASS = 16         # width classes (i-rows per core)
WIDTHS = [128 - 8 * t for t in range(NCLASS)]            # 128,120,...,8
OFFS = np.concatenate([[0], np.cumsum(WIDTHS)])          # class col offsets
NCOLS = int(OFFS[-1])                                    # 1088
# chunk = (class range); each chunk is one matmul (N<=512)
CHUNKS = [(0, 4), (4, 9), (9, 16)]
CHUNK_COLS = [int(OFFS[hi] - OFFS[lo]) for lo, hi in CHUNKS]  # 464, 400, 224
# pp DRAM layout: chunk-major, tiles paired: [c0t0|c0t1|c1t0|c1t1|c2t0|c2t1]
PP_OFFS = np.concatenate([[0], np.cumsum([2 * n for n in CHUNK_COLS])])

_CACHE = {}


def _build_nc():
    nc = bacc.Bacc("TRN2", target_bir_lowering=False, debug=False)
    # xw = xt (cols 0..255) | wd (cols 256..1343), bf16
    xw = nc.dram_tensor("xw", [P, 256 + NCOLS], BF16, kind="ExternalInput")
    pp = nc.dram_tensor("pp", [P, 2 * NCOLS], BF16, kind="ExternalInput")
    out = nc.dram_tensor("out", [2, P], F32, kind="ExternalOutput")

    with tile.TileContext(nc) as tc:
        with (
            tc.tile_pool(name="const", bufs=1) as cpool,
            tc.tile_pool(name="sb", bufs=1) as sb,
            tc.tile_pool(name="scr", bufs=2) as scr,
            tc.tile_pool(name="ps", bufs=1, space="PSUM") as ps,
            tc.tile_pool(name="pso", bufs=1, space="PSUM") as pso,
        ):
            xw_t = cpool.tile([P, 256 + NCOLS], BF16, tag="xw")
            nc.sync.dma_start(xw_t[:], xw[:])
            pp_t = cpool.tile([P, 2 * NCOLS], BF16, tag="pp")
            for ci in range(3):
                o, e = int(PP_OFFS[ci]), int(PP_OFFS[ci + 1])
                nc.sync.dma_start(pp_t[:, o:e], pp[:, o:e])

            id_t = cpool.tile([P, P], BF16, tag="idm")
            make_identity(nc, id_t[:])

            g = sb.tile([P, 6], F32, tag="g")
            # bf16 res2 -> PE transpose runs at 1 cycle/row instead of 2;
            # only rounds the final per-batch sums (~0.4% rel, gate is 2e-2)
            res2 = sb.tile([P, 2], BF16, tag="res2")
            # chunk-major: follow DMA arrival order
            for ci, (lo, hi) in enumerate(CHUNKS):
                n = CHUNK_COLS[ci]
                o = int(OFFS[lo])
                for bt in range(2):
                    z_ps = ps.tile([P, 464], F32, tag=f"z{ci}{bt}",
                                   name=f"z{ci}{bt}_ps")
                    nc.tensor.matmul(
                        z_ps[:, :n],
                        xw_t[:, bt * P:(bt + 1) * P],
                        xw_t[:, 256 + o:256 + o + n],
                        start=True, stop=True,
                    )
                    s = scr.tile([P, 464], F32, tag="s", name="s_t")
                    nc.vector.scalar_tensor_tensor(
                        out=s[:, :n],
                        in0=z_ps[:, :n],
                        scalar=1.0,
                        in1=pp_t[:, int(PP_OFFS[ci]) + bt * n:
                                 int(PP_OFFS[ci]) + (bt + 1) * n],
                        op0=mybir.AluOpType.mult,
                        op1=mybir.AluOpType.mult,
                        accum_out=g[:, 3 * bt + ci:3 * bt + ci + 1],
                    )
            with nc.allow_low_precision("final per-batch sums; 0.4% << 2e-2 gate"):
                for bt in range(2):
                    nc.vector.reduce_sum(
                        res2[:, bt:bt + 1], g[:, 3 * bt:3 * bt + 3],
                        axis=mybir.AxisListType.X,
                    )
            t_ps = pso.tile([2, P], BF16, tag="t")
            nc.tensor.transpose(t_ps[:], res2[:], id_t[:])
            t_sb = sb.tile([2, P], F32, tag="tsb")
            nc.vector.tensor_copy(t_sb[:], t_ps[:])
            nc.sync.dma_start(out[:], t_sb[:])
    nc.compile()
    return nc


def _prep_inputs(x, weights, comb_idx):
    """Host-side layout prep: xf paddings, per-core dense weight chunks, and
    the pair-product matrices P[b,(i,j)] = xf_i * xf_j (bf16)."""
    x = np.ascontiguousarray(np.asarray(x, dtype=np.float32))
    w = np.asarray(weights, dtype=np.float32).ravel()
    ci = np.asarray(comb_idx)
    i_, j_ = ci[:, 0].astype(np.int64), ci[:, 1].astype(np.int64)
    k_ = ci[:, 2].astype(np.int64)

    xf = np.concatenate([np.ones((B, 1), np.float32), x], axis=1)   # [256,127]
    xt = np.zeros((P, B), np.float32)
    xt[:NF, :] = xf.T
    xt_b = xt.astype(NP_BF16)

    # lex pair-row index of each combo
    ar = np.arange(NF, dtype=np.int64)
    rsp = ar * NF - (ar * (ar - 1)) // 2
    q = rsp[i_] + (j_ - i_)
    Wd = np.zeros((8128, NF), np.float32)
    Wd[q, k_] = w

    in_maps = []
    for c in range(8):
        big = np.zeros((P, NCOLS), np.float32)
        Pp = np.zeros((B, NCOLS), np.float32)
        for t in range(NCLASS):
            i = 8 * t + c
            if i > 126:
                continue
            o = int(OFFS[t])
            wdt = WIDTHS[t]
            p0 = int(rsp[i])
            # cols j in [i,127) hold Wd rows p0..p0+(127-i); leading j in
            # [8t, i) and trailing j=127 stay zero
            big[:NF, o + (i - 8 * t): o + (127 - 8 * t)] = Wd[p0:p0 + (NF - i)].T
            # pair products for j grid 8t..min(8t+w,127)-1 (zero-weight cols:
            # value irrelevant, z=0 there)
            jhi = min(8 * t + wdt, NF)
            Pp[:, o:o + (jhi - 8 * t)] = xf[:, i:i + 1] * xf[:, 8 * t:jhi]
        Pp_b = Pp.astype(NP_BF16)
        # chunk-major, tile-paired pp layout
        pieces = []
        for cidx, (lo, hi) in enumerate(CHUNKS):
            o, e = int(OFFS[lo]), int(OFFS[hi])
            pieces.append(Pp_b[:P, o:e])
            pieces.append(Pp_b[P:, o:e])
        m = {
            "xw": np.ascontiguousarray(
                np.concatenate([xt_b, big.astype(NP_BF16)], axis=1)),
            "pp": np.ascontiguousarray(np.concatenate(pieces, axis=1)),
        }
        in_maps.append(m)
    return in_maps


def _get_nc():
    if "nc" not in _CACHE:
        _CACHE["nc"] = _build_nc()
    return _CACHE["nc"]


def run_spmd(x, weights, comb_idx, trace=False):
    nc = _get_nc()
    in_maps = _prep_inputs(x, weights, comb_idx)
    res = run_bass_kernel_spmd(nc, in_maps, list(range(8)), trace=trace)
    acc = np.zeros(B, np.float64)
    for c in range(8):
        acc += res.results[c]["out"].astype(np.float64).ravel()
    return acc.astype(np.float32)[:, None], res


def kernel(x, weights, comb_idx):
    out, _ = run_spmd(x, weights, comb_idx, trace=False)
    return out
